# revision 18
# baseline (speedup 1.0000x reference)
"""Trainium2 Bass kernel for BigVGAN AMPBlock1 (B=4, T=8192, C=512, K=3,
dilations (1,3,5)), distributed over 8 NeuronCores.

Sharding: batch(4) x time(2), zero communication. Each core gets a time
slice with a 128-column halo on its interior side; reference padding
semantics (edge-replicate for resampling filters, zero for SAME convs)
are applied at both ends of every core's extended domain. The corruption
this causes at interior slice edges lands entirely in the discarded halo;
at true global edges it reproduces the reference exactly.

Per-core pipeline (channels-on-partitions layout, 4 chunks of 128):
  per dilation d: up2(x) [phase-decomposed 6-tap filters] -> snake
  [sin^2(ax)/a via cos identity, constant folded into the down output
  bias] -> down2 -> conv1(d) -> up2/snake/down2 -> conv2(1) -> +x.

Engines: dense convs and (configurable) filter taps run on TensorE as
accumulating matmuls (filters use scaled-identity stationary operands so
time-shifts come free via SBUF column offsets; the residual add is one
extra identity matmul into conv2's PSUM group). Remaining filter taps run
on VectorE as scalar_tensor_tensor MAC chains over full-chunk-width bf16
buffers. Snake = ScalarE Sin (bias pi/2 = cos) + one VectorE op.
fp32 residual; float32r matmuls where inputs are fp32; bf16 elsewhere.
"""

import sys

if "/opt/trn_rl_repo" not in sys.path:
    sys.path.insert(0, "/opt/trn_rl_repo")

import numpy as np

# ---------------------------------------------------------------- constants
RATIO = 2
K_FILT = 12
DILATIONS = (1, 3, 5)
B, T, C = 4, 8192, 512
NCHUNK = 4          # channel chunks of 128
H = 64              # halo (each side; one side per core is a global edge)
TP = 4096 + 2 * H   # 4224 extended time domain per core
PAD = 8             # pad columns on each side of stage buffers
W = TP + 2 * PAD    # 4240 buffer width
TW = [512] * 8 + [128]          # time-tile widths (sum = TP)
TOFF = [512 * i for i in range(9)]

# engine assignment per stage: for each chunk c (0..3), 'pe' or 'split'
# ('split' = even-offset taps on DVE full-width, odd taps + merge on PE)
CONFIG = {
    "up1":   ["pe"] * 4,
    "down1": ["split"] * 4,
    "up2":   ["split"] * 4,
    "down2": ["split"] * 4,
}
TAP_MIN = 0.003   # drop filter taps with |coef| below this (edge taps ~0.002)


def _filt():
    half_size = K_FILT // 2
    delta_f = 4.0 * (0.6 / RATIO)
    A = 2.285 * (half_size - 1) * np.pi * delta_f + 7.95
    beta = 0.1102 * (A - 8.7) if A > 50.0 else (
        0.5842 * (A - 21.0) ** 0.4 + 0.07886 * (A - 21.0) if A >= 21.0 else 0.0)
    window = np.kaiser(K_FILT, beta)
    time = np.arange(-half_size, half_size) + 0.5
    cutoff = 0.5 / RATIO
    filt = 2.0 * cutoff * window * np.sinc(2.0 * cutoff * time)
    return (filt / filt.sum()).astype(np.float64)


FILT = _filt()
FE = FILT[0::2]   # even taps
FO = FILT[1::2]   # odd taps

_PROG_CACHE = {}


def _build_program():
    import concourse.bass as bass
    import concourse.tile as tile
    from concourse import bacc, mybir

    f32 = mybir.dt.float32
    f32r = mybir.dt.float32r
    bf16 = mybir.dt.bfloat16
    AF = mybir.ActivationFunctionType
    OP = mybir.AluOpType

    nc = bacc.Bacc("TRN2", target_bir_lowering=False, debug=False, num_devices=8)

    # ------------------------------------------------------------- dram io
    xT = nc.dram_tensor("xT", [C, TP], f32r, kind="ExternalInput")
    # weights: per chunk ci: (128, iter*2conv*3tap*4co*128) bf16
    WCOLS = 3 * 2 * 3 * 4 * 128
    wk = nc.dram_tensor("wk", [NCHUNK, 128, WCOLS], bf16, kind="ExternalInput")
    # identities: f32 up-scaled (12: 2FE,2FO) + 1 plain; bf16 up (12) + down (12: FO,FE)
    idnf = nc.dram_tensor("idnf", [13, 128, 128], f32r, kind="ExternalInput")
    idnb = nc.dram_tensor("idnb", [25, 128, 128], bf16, kind="ExternalInput")
    # snake params per chunk: (128, 6 acts * 3 {2a, -c1, c1})
    snk = nc.dram_tensor("snk", [NCHUNK, 128, 20], f32, kind="ExternalInput")
    # conv biases per chunk: (128, 3 iters * 2 convs)
    bia = nc.dram_tensor("bia", [NCHUNK, 128, 6], f32, kind="ExternalInput")
    out = nc.dram_tensor("out", [C, TP], f32r, kind="ExternalOutput")

    with tile.TileContext(nc) as tc:
        with (
            tc.tile_pool(name="persist", bufs=1) as pp,
            tc.tile_pool(name="sphase", bufs=2) as sp,
            tc.tile_pool(name="wpool", bufs=1) as wp,
            tc.tile_pool(name="scratch", bufs=4) as scr,
            tc.tile_pool(name="rrpool", bufs=3) as rrp,
            tc.tile_pool(name="ripool", bufs=3) as rip,
            tc.tile_pool(name="psum", bufs=8, space="PSUM") as psp,
        ):
            # persistent buffers
            xb = [pp.tile([128, W], f32r, tag=f"xb{c}", name=f"xb{c}") for c in range(NCHUNK)]
            zb = [pp.tile([128, W], bf16, tag=f"zb{c}", name=f"zb{c}") for c in range(NCHUNK)]
            s2 = [pp.tile([128, W], bf16, tag=f"s2{c}", name=f"s2{c}") for c in range(NCHUNK)]
            idf = [pp.tile([128, 128], f32r, tag=f"idf{j}", name=f"idf{j}") for j in range(13)]
            idb = [pp.tile([128, 128], bf16, tag=f"idb{j}", name=f"idb{j}") for j in range(25)]
            snkt = [pp.tile([128, 20], f32, tag=f"snk{c}", name=f"snkt{c}") for c in range(NCHUNK)]
            biat = [pp.tile([128, 6], f32, tag=f"bia{c}", name=f"biat{c}") for c in range(NCHUNK)]
            # per-iteration weights (reloaded each iteration)
            wt = [wp.tile([128, 3 * 4 * 128], bf16, tag=f"w{c}", bufs=1, name=f"wt{c}")
                  for c in range(NCHUNK)]

            # ------------------------------------------------ initial DMAs
            for c in range(NCHUNK):
                nc.sync.dma_start(xb[c][:, PAD:PAD + TP], xT[c * 128:(c + 1) * 128, :])
            for j in range(13):
                nc.sync.dma_start(idf[j][:], idnf[j])
            for j in range(25):
                nc.sync.dma_start(idb[j][:], idnb[j])
            for c in range(NCHUNK):
                nc.sync.dma_start(snkt[c][:], snk[c])
                nc.sync.dma_start(biat[c][:], bia[c])
                # zero pads of z buffer (stay zero forever: SAME conv padding)
                nc.vector.memset(zb[c][:, 0:PAD], 0.0)
                nc.vector.memset(zb[c][:, PAD + TP:W], 0.0)

            def bcast(buf, dst_lo, dst_hi, src_col):
                src = buf[:, src_col:src_col + 1].broadcast_to((128, dst_hi - dst_lo))
                nc.vector.tensor_copy(buf[:, dst_lo:dst_hi], src)

            def edge_pads(buf):
                bcast(buf, 0, PAD, PAD)
                bcast(buf, PAD + TP, W, PAD + TP - 1)

            for c in range(NCHUNK):
                edge_pads(xb[c])

            # helper: column of snake/bias params
            def snk_col(c, act, which):  # which: 0=2a, 1=-c1, 2=c1
                return snkt[c][:, act * 3 + which:act * 3 + which + 1]

            def bia_col(c, it, conv):
                return biat[c][:, it * 2 + conv:it * 2 + conv + 1]

            def rr_col(c):   # 0.25 + 512.5 (range-reduction cast shift)
                return snkt[c][:, 18:19]

            def negpi_col(c):
                return snkt[c][:, 19:20]

            TWO_PI = float(np.float32(2.0 * np.pi))

            def snake_tile(y_ap, dst_ap, c, aopi, negc1, w0):
                """dst = y - c1*cos(2a*y); y_ap may be PSUM(f32) or SBUF(bf16).
                cos(2a*y) = sin(v), v = 2a*y + pi/2. Range reduction:
                t = v/2pi + 512 = y*(a/pi) + 512.25;  i = hw round-to-nearest
                cast;  frac = t - i in [-0.5, 0.5];  sin(v) = sin(2pi*frac)."""
                rr = rrp.tile([128, w0], f32, tag="rr", name="rr")
                nc.scalar.activation(rr[:], y_ap, AF.Identity,
                                     bias=rr_col(c), scale=aopi)
                ri = rip.tile([128, w0], mybir.dt.int16, tag="ri", name="ri")
                nc.vector.tensor_copy(ri[:], rr[:])
                nc.vector.scalar_tensor_tensor(rr[:], ri[:], -1.0, rr[:],
                                               OP.mult, OP.add)
                cos_t = scr.tile([128, w0], bf16, tag="cos", name="cos_t")
                nc.scalar.activation(cos_t[:], rr[:], AF.Sin,
                                     bias=0.0, scale=TWO_PI)
                nc.vector.scalar_tensor_tensor(
                    dst_ap, cos_t[:], negc1, y_ap, OP.mult, OP.add)

            # ---------------------------------------------------- stages
            # tap tables: (idx into identity array, coef, delta)
            def _up_taps(phase):
                coefs = FE if phase == 0 else FO
                base = -3 if phase == 0 else -2
                return [(phase * 6 + r, 2.0 * float(coefs[r]), base + r)
                        for r in range(6) if abs(coefs[r]) >= TAP_MIN]

            def _down_taps():
                out = []
                for srci, (coefs, base, idoff) in enumerate(
                        ((FO, -2, 12), (FE, -3, 18))):
                    for r in range(6):
                        if abs(coefs[r]) >= TAP_MIN:
                            out.append((srci, idoff + r, float(coefs[r]), base + r))
                return out

            def up_snake(src, sebuf, sobuf, c, act, src_f32):
                """up2 + snake for chunk c: src -> sebuf/sobuf (bf16).
                mode 'pe': all taps as PE scaled-identity matmuls.
                mode 'split': even-offset taps accumulate on DVE (full width)
                into dst, then PE adds them into PSUM via a plain-identity
                matmul alongside the odd-offset taps."""
                ids = idf if src_f32 else idb
                mode = CONFIG["up1" if src_f32 else "up2"][c]
                aopi = snk_col(c, act, 0)
                negc1 = snk_col(c, act, 1)
                for phase, (dst, base_off) in enumerate(((sebuf, -3), (sobuf, -2))):
                    taps = _up_taps(phase)
                    if mode == "split":
                        dve = [tp for tp in taps if tp[2] % 2 == 0]
                        pe = [tp for tp in taps if tp[2] % 2 != 0]
                    else:
                        dve, pe = [], taps
                    dsl = dst[:, PAD:PAD + TP]
                    for j, (_, coef, dl) in enumerate(dve):
                        sl = src[:, PAD + dl:PAD + dl + TP]
                        if j == 0:
                            nc.vector.tensor_scalar(dsl, sl, coef, None, OP.mult)
                        else:
                            nc.vector.scalar_tensor_tensor(
                                dsl, sl, coef, dsl, OP.mult, OP.add)
                    for t in range(9):
                        w0, o0 = TW[t], TOFF[t]
                        ps = psp.tile([128, w0], mybir.dt.float32, tag="ps", name="ps")
                        n = 0
                        ntot = len(pe) + (1 if dve else 0)
                        if dve:  # merge DVE partial via plain identity
                            nc.tensor.matmul(ps[:], idb[24][:],
                                             dst[:, PAD + o0:PAD + o0 + w0],
                                             start=True, stop=(ntot == 1))
                            n = 1
                        for (idi, _, dl) in pe:
                            rhs = src[:, PAD + o0 + dl:PAD + o0 + dl + w0]
                            nc.tensor.matmul(ps[:], ids[idi][:], rhs,
                                             start=(n == 0), stop=(n == ntot - 1))
                            n += 1
                        snake_tile(ps[:], dst[:, PAD + o0:PAD + o0 + w0],
                                   c, aopi, negc1, w0)

            def phase_pads(sebuf, sobuf):
                # left pads (both phases) = se[0]; right pads (both) = so[last]
                bcast(sebuf, 0, PAD, PAD)
                src = sebuf[:, PAD:PAD + 1].broadcast_to((128, PAD))
                nc.vector.tensor_copy(sobuf[:, 0:PAD], src)
                src = sobuf[:, PAD + TP - 1:PAD + TP].broadcast_to((128, PAD))
                nc.vector.tensor_copy(sebuf[:, PAD + TP:W], src)
                bcast(sobuf, PAD + TP, W, PAD + TP - 1)

            def down(sebuf, sobuf, dstbuf, c, act, key):
                """down2: se/so -> dstbuf (bf16), + c1 constant.
                'split': even-offset taps on DVE accumulate (with +c1 folded
                into the first op) into dstbuf; PE adds them via plain
                identity + odd taps in PSUM; ACT copies back (bias 0)."""
                mode = CONFIG[key][c]
                c1 = snk_col(c, act, 2)
                srcs = (sebuf, sobuf)
                taps = _down_taps()
                if mode == "split":
                    dve = [tp for tp in taps if tp[3] % 2 == 0]
                    pe = [tp for tp in taps if tp[3] % 2 != 0]
                else:
                    dve, pe = [], taps
                dsl = dstbuf[:, PAD:PAD + TP]
                for j, (srci, _, coef, dl) in enumerate(dve):
                    sl = srcs[srci][:, PAD + dl:PAD + dl + TP]
                    if j == 0:
                        nc.vector.tensor_scalar(dsl, sl, coef, c1, OP.mult, OP.add)
                    else:
                        nc.vector.scalar_tensor_tensor(
                            dsl, sl, coef, dsl, OP.mult, OP.add)
                for t in range(9):
                    w0, o0 = TW[t], TOFF[t]
                    ps = psp.tile([128, w0], mybir.dt.float32, tag="ps", name="ps")
                    n = 0
                    ntot = len(pe) + (1 if dve else 0)
                    if dve:
                        nc.tensor.matmul(ps[:], idb[24][:],
                                         dstbuf[:, PAD + o0:PAD + o0 + w0],
                                         start=True, stop=(ntot == 1))
                        n = 1
                    for (srci, idi, _, dl) in pe:
                        nc.tensor.matmul(
                            ps[:], idb[idi][:],
                            srcs[srci][:, PAD + o0 + dl:PAD + o0 + dl + w0],
                            start=(n == 0), stop=(n == ntot - 1))
                        n += 1
                    nc.scalar.activation(
                        dstbuf[:, PAD + o0:PAD + o0 + w0], ps[:], AF.Identity,
                        bias=(0.0 if dve else c1), scale=1.0)

            def conv2_(srcbufs, it, cv, d, residual):
                """conv K=3 dilation d over bf16 srcbufs; writes s2 (cv=0) or
                xb (cv=1, fp32 with extra residual identity tap)."""
                for ci in range(NCHUNK):
                    nc.sync.dma_start(
                        wt[ci][:],
                        wk[ci, :, (it * 2 + cv) * 1536:(it * 2 + cv + 1) * 1536])
                for co in range(NCHUNK):
                    for t in range(9):
                        w0, o0 = TW[t], TOFF[t]
                        ps = psp.tile([128, w0], mybir.dt.float32, tag="ps", name="ps")
                        mms = []
                        for k in range(3):
                            off = (k - 1) * d
                            for ci in range(NCHUNK):
                                wcol = (k * 4 + co) * 128
                                mms.append((wt[ci][:, wcol:wcol + 128],
                                            srcbufs[ci][:, PAD + o0 + off:
                                                        PAD + o0 + off + w0]))
                        last = len(mms) - (0 if residual else 1)
                        for j, (lhs, rhs) in enumerate(mms):
                            nc.tensor.matmul(ps[:], lhs, rhs,
                                             start=(j == 0),
                                             stop=(not residual and j == last))
                        if residual:
                            nc.tensor.matmul(
                                ps[:], idf[12][:],
                                xb[co][:, PAD + o0:PAD + o0 + w0],
                                start=False, stop=True)
                            dst = xb[co][:, PAD + o0:PAD + o0 + w0]
                        else:
                            dst = s2[co][:, PAD + o0:PAD + o0 + w0]
                        nc.scalar.activation(dst, ps[:], AF.Identity,
                                             bias=bia_col(co, it, cv), scale=1.0)

            # ---------------------------------------------------- main loop
            for it, d in enumerate(DILATIONS):
                # act1d #1: up+snake, pads, down -> zb
                for c in range(NCHUNK):
                    sebuf = sp.tile([128, W], bf16, tag="se", name="sebuf")
                    sobuf = sp.tile([128, W], bf16, tag="so", name="sobuf")
                    up_snake(xb[c], sebuf, sobuf, c, 2 * it, True)
                    phase_pads(sebuf, sobuf)
                    down(sebuf, sobuf, zb[c], c, 2 * it, "down1")
                # conv1 -> s2
                conv2_(zb, it, 0, d, residual=False)
                for c in range(NCHUNK):
                    edge_pads(s2[c])
                # act1d #2 -> zb (reused)
                for c in range(NCHUNK):
                    sebuf = sp.tile([128, W], bf16, tag="se", name="sebuf")
                    sobuf = sp.tile([128, W], bf16, tag="so", name="sobuf")
                    up_snake(s2[c], sebuf, sobuf, c, 2 * it + 1, False)
                    phase_pads(sebuf, sobuf)
                    down(sebuf, sobuf, zb[c], c, 2 * it + 1, "down2")
                # conv2 + residual -> xb
                conv2_(zb, it, 1, 1, residual=True)
                for c in range(NCHUNK):
                    edge_pads(xb[c])

            # ------------------------------------------------------ output
            for c in range(NCHUNK):
                nc.sync.dma_start(out[c * 128:(c + 1) * 128, :],
                                  xb[c][:, PAD:PAD + TP])

    nc.compile()
    return nc


def _host_prep(x, v1, g1, b1, v2, g2, b2, alphas):
    import ml_dtypes

    def fold(v, g):
        o = np.empty_like(v, dtype=np.float64)
        for i in range(v.shape[0]):
            norm = np.linalg.norm(v[i].reshape(-1, v.shape[-1]).astype(np.float64),
                                  axis=0)
            o[i] = v[i] / norm[None, None, :] * g[i][0].astype(np.float64)
        return o

    kern1 = fold(np.asarray(v1), np.asarray(g1))   # (3, 3, C, C) (k, ci, co)
    kern2 = fold(np.asarray(v2), np.asarray(g2))

    # weights layout: wk[ci_chunk, 128, (it*2+cv)*1536 + (k*4+co)*128 + q]
    WCOLS = 3 * 2 * 3 * 4 * 128
    wk = np.zeros((NCHUNK, 128, WCOLS), dtype=np.float64)
    for it in range(3):
        for cv, kern in ((0, kern1), (1, kern2)):
            for k in range(3):
                for ci in range(NCHUNK):
                    for co in range(NCHUNK):
                        col = (it * 2 + cv) * 1536 + (k * 4 + co) * 128
                        wk[ci, :, col:col + 128] = \
                            kern[it][k, ci * 128:(ci + 1) * 128,
                                     co * 128:(co + 1) * 128]
    wk = wk.astype(ml_dtypes.bfloat16)

    eye = np.eye(128, dtype=np.float64)
    idnf = np.zeros((13, 128, 128), dtype=np.float64)
    for r in range(6):
        idnf[r] = 2.0 * FE[r] * eye
        idnf[6 + r] = 2.0 * FO[r] * eye
    idnf[12] = eye
    idnb = np.zeros((25, 128, 128), dtype=np.float64)
    idnb[24] = eye
    for r in range(6):
        idnb[r] = 2.0 * FE[r] * eye      # up even
        idnb[6 + r] = 2.0 * FO[r] * eye  # up odd
        idnb[12 + r] = FO[r] * eye       # down, se taps
        idnb[18 + r] = FE[r] * eye       # down, so taps
    idnf = idnf.astype(np.float32)
    idnb = idnb.astype(ml_dtypes.bfloat16)

    al = np.asarray(alphas, dtype=np.float64)      # (6, C) log alpha
    a = np.exp(al)
    c1 = 0.5 / (a + 1e-9)
    snk = np.zeros((NCHUNK, 128, 20), dtype=np.float32)
    snk[:, :, 18] = np.float32(0.25 + 512.0)
    snk[:, :, 19] = np.float32(-np.pi)
    for c in range(NCHUNK):
        for act in range(6):
            snk[c, :, act * 3 + 0] = a[act, c * 128:(c + 1) * 128] / np.pi
            snk[c, :, act * 3 + 1] = -c1[act, c * 128:(c + 1) * 128]
            snk[c, :, act * 3 + 2] = c1[act, c * 128:(c + 1) * 128]

    bia = np.zeros((NCHUNK, 128, 6), dtype=np.float32)
    b1a, b2a = np.asarray(b1), np.asarray(b2)
    for c in range(NCHUNK):
        for it in range(3):
            bia[c, :, it * 2 + 0] = b1a[it, c * 128:(c + 1) * 128]
            bia[c, :, it * 2 + 1] = b2a[it, c * 128:(c + 1) * 128]

    xa = np.asarray(x)
    in_maps = []
    for b in range(B):
        for h in range(2):
            sl = xa[b, 0:TP, :] if h == 0 else xa[b, T - TP:T, :]
            xTc = np.ascontiguousarray(sl.T.astype(np.float32))  # (C, TP)
            in_maps.append({
                "xT": xTc, "wk": wk, "idnf": idnf, "idnb": idnb,
                "snk": snk, "bia": bia,
            })
    return in_maps


def _assemble(results):
    out = np.empty((B, T, C), dtype=np.float32)
    i = 0
    for b in range(B):
        for h in range(2):
            oT = results[i]["out"]          # (C, TP)
            o = np.ascontiguousarray(oT.T)  # (TP, C)
            if h == 0:
                out[b, 0:4096] = o[0:4096]
            else:
                out[b, 4096:T] = o[2 * H:TP]
            i += 1
    return out


def kernel(x, v1, g1, b1, v2, g2, b2, alphas):
    from concourse.bass_utils import run_bass_kernel_spmd

    key = "prog"
    if key not in _PROG_CACHE:
        _PROG_CACHE[key] = _build_program()
    nc = _PROG_CACHE[key]
    in_maps = _host_prep(x, v1, g1, b1, v2, g2, b2, alphas)
    res = run_bass_kernel_spmd(nc, in_maps, core_ids=list(range(8)))
    return _assemble(res.results)


# revision 19
# speedup vs baseline: 1.4679x; 1.4679x over previous
"""Trainium2 Bass kernel for BigVGAN AMPBlock1 (B=4, T=8192, C=512, K=3,
dilations (1,3,5)), distributed over 8 NeuronCores.

Sharding: batch(4) x time(2), zero communication. Each core gets a time
slice with a 128-column halo on its interior side; reference padding
semantics (edge-replicate for resampling filters, zero for SAME convs)
are applied at both ends of every core's extended domain. The corruption
this causes at interior slice edges lands entirely in the discarded halo;
at true global edges it reproduces the reference exactly.

Per-core pipeline (channels-on-partitions layout, 4 chunks of 128):
  per dilation d: up2(x) [phase-decomposed 6-tap filters] -> snake
  [sin^2(ax)/a via cos identity, constant folded into the down output
  bias] -> down2 -> conv1(d) -> up2/snake/down2 -> conv2(1) -> +x.

Engines: dense convs and (configurable) filter taps run on TensorE as
accumulating matmuls (filters use scaled-identity stationary operands so
time-shifts come free via SBUF column offsets; the residual add is one
extra identity matmul into conv2's PSUM group). Remaining filter taps run
on VectorE as scalar_tensor_tensor MAC chains over full-chunk-width bf16
buffers. Snake = ScalarE Sin (bias pi/2 = cos) + one VectorE op.
fp32 residual; float32r matmuls where inputs are fp32; bf16 elsewhere.
"""

import sys

if "/opt/trn_rl_repo" not in sys.path:
    sys.path.insert(0, "/opt/trn_rl_repo")

import numpy as np

# ---------------------------------------------------------------- constants
RATIO = 2
K_FILT = 12
DILATIONS = (1, 3, 5)
B, T, C = 4, 8192, 512
NCHUNK = 4          # channel chunks of 128
H = 64              # halo (each side; one side per core is a global edge)
TP = 4096 + 2 * H   # 4224 extended time domain per core
PAD = 8             # pad columns on each side of stage buffers
W = TP + 2 * PAD    # 4240 buffer width
TW = [512] * 8 + [128]          # time-tile widths (sum = TP)
TOFF = [512 * i for i in range(9)]

# engine assignment per stage: for each chunk c (0..3), 'pe' or 'split'
# ('split' = even-offset taps on DVE full-width, odd taps + merge on PE)
CONFIG = {
    "up1":   ["pe"] * 4,
    "down1": ["pe"] * 4,
    "up2":   ["pe"] * 4,
    "down2": ["pe"] * 4,
}
TAP_MIN = 0.003   # drop filter taps with |coef| below this (edge taps ~0.002)


def _filt():
    half_size = K_FILT // 2
    delta_f = 4.0 * (0.6 / RATIO)
    A = 2.285 * (half_size - 1) * np.pi * delta_f + 7.95
    beta = 0.1102 * (A - 8.7) if A > 50.0 else (
        0.5842 * (A - 21.0) ** 0.4 + 0.07886 * (A - 21.0) if A >= 21.0 else 0.0)
    window = np.kaiser(K_FILT, beta)
    time = np.arange(-half_size, half_size) + 0.5
    cutoff = 0.5 / RATIO
    filt = 2.0 * cutoff * window * np.sinc(2.0 * cutoff * time)
    return (filt / filt.sum()).astype(np.float64)


FILT = _filt()
FE = FILT[0::2]   # even taps
FO = FILT[1::2]   # odd taps

_PROG_CACHE = {}


def _build_program():
    import concourse.bass as bass
    import concourse.tile as tile
    from concourse import bacc, mybir

    f32 = mybir.dt.float32
    f32r = mybir.dt.float32r
    bf16 = mybir.dt.bfloat16
    AF = mybir.ActivationFunctionType
    OP = mybir.AluOpType

    nc = bacc.Bacc("TRN2", target_bir_lowering=False, debug=False, num_devices=8)

    # ------------------------------------------------------------- dram io
    xT = nc.dram_tensor("xT", [C, TP], f32r, kind="ExternalInput")
    # weights: per chunk ci: (128, iter*2conv*3tap*4co*128) bf16
    WCOLS = 3 * 2 * 3 * 4 * 128
    wk = nc.dram_tensor("wk", [NCHUNK, 128, WCOLS], bf16, kind="ExternalInput")
    # identities: f32 up-scaled (12: 2FE,2FO) + 1 plain; bf16 up (12) + down (12: FO,FE)
    idnf = nc.dram_tensor("idnf", [13, 128, 128], f32r, kind="ExternalInput")
    idnb = nc.dram_tensor("idnb", [25, 128, 128], bf16, kind="ExternalInput")
    # snake params per chunk: (128, 6 acts * 3 {2a, -c1, c1})
    snk = nc.dram_tensor("snk", [NCHUNK, 128, 20], f32, kind="ExternalInput")
    # conv biases per chunk: (128, 3 iters * 2 convs)
    bia = nc.dram_tensor("bia", [NCHUNK, 128, 6], f32, kind="ExternalInput")
    out = nc.dram_tensor("out", [C, TP], f32r, kind="ExternalOutput")

    with tile.TileContext(nc) as tc:
        with (
            tc.tile_pool(name="persist", bufs=1) as pp,
            tc.tile_pool(name="sphase", bufs=2) as sp,
            tc.tile_pool(name="wpool", bufs=1) as wp,
            tc.tile_pool(name="scratch", bufs=4) as scr,
            tc.tile_pool(name="rrpool", bufs=3) as rrp,
            tc.tile_pool(name="ripool", bufs=3) as rip,
            tc.tile_pool(name="psum", bufs=8, space="PSUM") as psp,
        ):
            # persistent buffers
            xb = [pp.tile([128, W], f32r, tag=f"xb{c}", name=f"xb{c}") for c in range(NCHUNK)]
            zb = [pp.tile([128, W], bf16, tag=f"zb{c}", name=f"zb{c}") for c in range(NCHUNK)]
            s2 = [pp.tile([128, W], bf16, tag=f"s2{c}", name=f"s2{c}") for c in range(NCHUNK)]
            idf = [pp.tile([128, 128], f32r, tag=f"idf{j}", name=f"idf{j}") for j in range(13)]
            idb = [pp.tile([128, 128], bf16, tag=f"idb{j}", name=f"idb{j}") for j in range(25)]
            snkt = [pp.tile([128, 20], f32, tag=f"snk{c}", name=f"snkt{c}") for c in range(NCHUNK)]
            biat = [pp.tile([128, 6], f32, tag=f"bia{c}", name=f"biat{c}") for c in range(NCHUNK)]
            # per-iteration weights (reloaded each iteration)
            wt = [wp.tile([128, 3 * 4 * 128], bf16, tag=f"w{c}", bufs=1, name=f"wt{c}")
                  for c in range(NCHUNK)]

            # ------------------------------------------------ initial DMAs
            for c in range(NCHUNK):
                nc.sync.dma_start(xb[c][:, PAD:PAD + TP], xT[c * 128:(c + 1) * 128, :])
            for j in range(13):
                nc.sync.dma_start(idf[j][:], idnf[j])
            for j in range(25):
                nc.sync.dma_start(idb[j][:], idnb[j])
            for c in range(NCHUNK):
                nc.sync.dma_start(snkt[c][:], snk[c])
                nc.sync.dma_start(biat[c][:], bia[c])
                # zero pads of z buffer (stay zero forever: SAME conv padding)
                nc.vector.memset(zb[c][:, 0:PAD], 0.0)
                nc.vector.memset(zb[c][:, PAD + TP:W], 0.0)

            def bcast(buf, dst_lo, dst_hi, src_col):
                src = buf[:, src_col:src_col + 1].broadcast_to((128, dst_hi - dst_lo))
                nc.vector.tensor_copy(buf[:, dst_lo:dst_hi], src)

            def edge_pads(buf):
                bcast(buf, 0, PAD, PAD)
                bcast(buf, PAD + TP, W, PAD + TP - 1)

            for c in range(NCHUNK):
                edge_pads(xb[c])

            # helper: column of snake/bias params
            def snk_col(c, act, which):  # which: 0=2a, 1=-c1, 2=c1
                return snkt[c][:, act * 3 + which:act * 3 + which + 1]

            def bia_col(c, it, conv):
                return biat[c][:, it * 2 + conv:it * 2 + conv + 1]

            def rr_col(c):   # 0.25 + 512.5 (range-reduction cast shift)
                return snkt[c][:, 18:19]

            def negpi_col(c):
                return snkt[c][:, 19:20]

            TWO_PI = float(np.float32(2.0 * np.pi))

            def snake_tile(y_ap, dst_ap, c, aopi, negc1, w0):
                """dst = y - c1*cos(2a*y); y_ap may be PSUM(f32) or SBUF(bf16).
                cos(2a*y) = sin(v), v = 2a*y + pi/2. Range reduction:
                t = v/2pi + 512 = y*(a/pi) + 512.25;  i = hw round-to-nearest
                cast;  frac = t - i in [-0.5, 0.5];  sin(v) = sin(2pi*frac)."""
                rr = rrp.tile([128, w0], f32, tag="rr", name="rr")
                nc.scalar.activation(rr[:], y_ap, AF.Identity,
                                     bias=rr_col(c), scale=aopi)
                ri = rip.tile([128, w0], mybir.dt.int16, tag="ri", name="ri")
                nc.vector.tensor_copy(ri[:], rr[:])
                nc.vector.scalar_tensor_tensor(rr[:], ri[:], -1.0, rr[:],
                                               OP.mult, OP.add)
                cos_t = scr.tile([128, w0], bf16, tag="cos", name="cos_t")
                nc.scalar.activation(cos_t[:], rr[:], AF.Sin,
                                     bias=0.0, scale=TWO_PI)
                nc.vector.scalar_tensor_tensor(
                    dst_ap, cos_t[:], negc1, y_ap, OP.mult, OP.add)

            # ---------------------------------------------------- stages
            # tap tables: (idx into identity array, coef, delta)
            def _up_taps(phase):
                coefs = FE if phase == 0 else FO
                base = -3 if phase == 0 else -2
                return [(phase * 6 + r, 2.0 * float(coefs[r]), base + r)
                        for r in range(6) if abs(coefs[r]) >= TAP_MIN]

            def _down_taps():
                out = []
                for srci, (coefs, base, idoff) in enumerate(
                        ((FO, -2, 12), (FE, -3, 18))):
                    for r in range(6):
                        if abs(coefs[r]) >= TAP_MIN:
                            out.append((srci, idoff + r, float(coefs[r]), base + r))
                return out

            def up_snake(src, sebuf, sobuf, c, act, src_f32):
                """up2 + snake for chunk c: src -> sebuf/sobuf (bf16).
                mode 'pe': all taps as PE scaled-identity matmuls.
                mode 'split': even-offset taps accumulate on DVE (full width)
                into dst, then PE adds them into PSUM via a plain-identity
                matmul alongside the odd-offset taps."""
                ids = idf if src_f32 else idb
                mode = CONFIG["up1" if src_f32 else "up2"][c]
                aopi = snk_col(c, act, 0)
                negc1 = snk_col(c, act, 1)
                for phase, (dst, base_off) in enumerate(((sebuf, -3), (sobuf, -2))):
                    taps = _up_taps(phase)
                    if mode == "split":
                        dve = [tp for tp in taps if tp[2] % 2 == 0]
                        pe = [tp for tp in taps if tp[2] % 2 != 0]
                    else:
                        dve, pe = [], taps
                    dsl = dst[:, PAD:PAD + TP]
                    for j, (_, coef, dl) in enumerate(dve):
                        sl = src[:, PAD + dl:PAD + dl + TP]
                        if j == 0:
                            nc.vector.tensor_scalar(dsl, sl, coef, None, OP.mult)
                        else:
                            nc.vector.scalar_tensor_tensor(
                                dsl, sl, coef, dsl, OP.mult, OP.add)
                    for t in range(9):
                        w0, o0 = TW[t], TOFF[t]
                        ps = psp.tile([128, w0], mybir.dt.float32, tag="ps", name="ps")
                        n = 0
                        ntot = len(pe) + (1 if dve else 0)
                        if dve:  # merge DVE partial via plain identity
                            nc.tensor.matmul(ps[:], idb[24][:],
                                             dst[:, PAD + o0:PAD + o0 + w0],
                                             start=True, stop=(ntot == 1))
                            n = 1
                        for (idi, _, dl) in pe:
                            rhs = src[:, PAD + o0 + dl:PAD + o0 + dl + w0]
                            nc.tensor.matmul(ps[:], ids[idi][:], rhs,
                                             start=(n == 0), stop=(n == ntot - 1))
                            n += 1
                        snake_tile(ps[:], dst[:, PAD + o0:PAD + o0 + w0],
                                   c, aopi, negc1, w0)

            def phase_pads(sebuf, sobuf):
                # left pads (both phases) = se[0]; right pads (both) = so[last]
                bcast(sebuf, 0, PAD, PAD)
                src = sebuf[:, PAD:PAD + 1].broadcast_to((128, PAD))
                nc.vector.tensor_copy(sobuf[:, 0:PAD], src)
                src = sobuf[:, PAD + TP - 1:PAD + TP].broadcast_to((128, PAD))
                nc.vector.tensor_copy(sebuf[:, PAD + TP:W], src)
                bcast(sobuf, PAD + TP, W, PAD + TP - 1)

            def down(sebuf, sobuf, dstbuf, c, act, key):
                """down2: se/so -> dstbuf (bf16), + c1 constant.
                'split': even-offset taps on DVE accumulate (with +c1 folded
                into the first op) into dstbuf; PE adds them via plain
                identity + odd taps in PSUM; ACT copies back (bias 0)."""
                mode = CONFIG[key][c]
                c1 = snk_col(c, act, 2)
                srcs = (sebuf, sobuf)
                taps = _down_taps()
                if mode == "split":
                    dve = [tp for tp in taps if tp[3] % 2 == 0]
                    pe = [tp for tp in taps if tp[3] % 2 != 0]
                else:
                    dve, pe = [], taps
                dsl = dstbuf[:, PAD:PAD + TP]
                for j, (srci, _, coef, dl) in enumerate(dve):
                    sl = srcs[srci][:, PAD + dl:PAD + dl + TP]
                    if j == 0:
                        nc.vector.tensor_scalar(dsl, sl, coef, c1, OP.mult, OP.add)
                    else:
                        nc.vector.scalar_tensor_tensor(
                            dsl, sl, coef, dsl, OP.mult, OP.add)
                for t in range(9):
                    w0, o0 = TW[t], TOFF[t]
                    ps = psp.tile([128, w0], mybir.dt.float32, tag="ps", name="ps")
                    n = 0
                    ntot = len(pe) + (1 if dve else 0)
                    if dve:
                        nc.tensor.matmul(ps[:], idb[24][:],
                                         dstbuf[:, PAD + o0:PAD + o0 + w0],
                                         start=True, stop=(ntot == 1))
                        n = 1
                    for (srci, idi, _, dl) in pe:
                        nc.tensor.matmul(
                            ps[:], idb[idi][:],
                            srcs[srci][:, PAD + o0 + dl:PAD + o0 + dl + w0],
                            start=(n == 0), stop=(n == ntot - 1))
                        n += 1
                    nc.scalar.activation(
                        dstbuf[:, PAD + o0:PAD + o0 + w0], ps[:], AF.Identity,
                        bias=(0.0 if dve else c1), scale=1.0)

            def conv2_(srcbufs, it, cv, d, residual):
                """conv K=3 dilation d over bf16 srcbufs; writes s2 (cv=0) or
                xb (cv=1, fp32 with extra residual identity tap)."""
                for ci in range(NCHUNK):
                    nc.sync.dma_start(
                        wt[ci][:],
                        wk[ci, :, (it * 2 + cv) * 1536:(it * 2 + cv + 1) * 1536])
                for co in range(NCHUNK):
                    for t in range(9):
                        w0, o0 = TW[t], TOFF[t]
                        ps = psp.tile([128, w0], mybir.dt.float32, tag="ps", name="ps")
                        mms = []
                        for k in range(3):
                            off = (k - 1) * d
                            for ci in range(NCHUNK):
                                wcol = (k * 4 + co) * 128
                                mms.append((wt[ci][:, wcol:wcol + 128],
                                            srcbufs[ci][:, PAD + o0 + off:
                                                        PAD + o0 + off + w0]))
                        last = len(mms) - (0 if residual else 1)
                        for j, (lhs, rhs) in enumerate(mms):
                            nc.tensor.matmul(ps[:], lhs, rhs,
                                             start=(j == 0),
                                             stop=(not residual and j == last))
                        if residual:
                            nc.tensor.matmul(
                                ps[:], idf[12][:],
                                xb[co][:, PAD + o0:PAD + o0 + w0],
                                start=False, stop=True)
                            dst = xb[co][:, PAD + o0:PAD + o0 + w0]
                        else:
                            dst = s2[co][:, PAD + o0:PAD + o0 + w0]
                        nc.scalar.activation(dst, ps[:], AF.Identity,
                                             bias=bia_col(co, it, cv), scale=1.0)

            # ---------------------------------------------------- main loop
            for it, d in enumerate(DILATIONS):
                # act1d #1: up+snake, pads, down -> zb
                for c in range(NCHUNK):
                    sebuf = sp.tile([128, W], bf16, tag="se", name="sebuf")
                    sobuf = sp.tile([128, W], bf16, tag="so", name="sobuf")
                    up_snake(xb[c], sebuf, sobuf, c, 2 * it, True)
                    phase_pads(sebuf, sobuf)
                    down(sebuf, sobuf, zb[c], c, 2 * it, "down1")
                # conv1 -> s2
                conv2_(zb, it, 0, d, residual=False)
                for c in range(NCHUNK):
                    edge_pads(s2[c])
                # act1d #2 -> zb (reused)
                for c in range(NCHUNK):
                    sebuf = sp.tile([128, W], bf16, tag="se", name="sebuf")
                    sobuf = sp.tile([128, W], bf16, tag="so", name="sobuf")
                    up_snake(s2[c], sebuf, sobuf, c, 2 * it + 1, False)
                    phase_pads(sebuf, sobuf)
                    down(sebuf, sobuf, zb[c], c, 2 * it + 1, "down2")
                # conv2 + residual -> xb
                conv2_(zb, it, 1, 1, residual=True)
                for c in range(NCHUNK):
                    edge_pads(xb[c])

            # ------------------------------------------------------ output
            for c in range(NCHUNK):
                nc.sync.dma_start(out[c * 128:(c + 1) * 128, :],
                                  xb[c][:, PAD:PAD + TP])

    nc.compile()
    return nc


def _host_prep(x, v1, g1, b1, v2, g2, b2, alphas):
    import ml_dtypes

    def fold(v, g):
        o = np.empty_like(v, dtype=np.float64)
        for i in range(v.shape[0]):
            norm = np.linalg.norm(v[i].reshape(-1, v.shape[-1]).astype(np.float64),
                                  axis=0)
            o[i] = v[i] / norm[None, None, :] * g[i][0].astype(np.float64)
        return o

    kern1 = fold(np.asarray(v1), np.asarray(g1))   # (3, 3, C, C) (k, ci, co)
    kern2 = fold(np.asarray(v2), np.asarray(g2))

    # weights layout: wk[ci_chunk, 128, (it*2+cv)*1536 + (k*4+co)*128 + q]
    WCOLS = 3 * 2 * 3 * 4 * 128
    wk = np.zeros((NCHUNK, 128, WCOLS), dtype=np.float64)
    for it in range(3):
        for cv, kern in ((0, kern1), (1, kern2)):
            for k in range(3):
                for ci in range(NCHUNK):
                    for co in range(NCHUNK):
                        col = (it * 2 + cv) * 1536 + (k * 4 + co) * 128
                        wk[ci, :, col:col + 128] = \
                            kern[it][k, ci * 128:(ci + 1) * 128,
                                     co * 128:(co + 1) * 128]
    wk = wk.astype(ml_dtypes.bfloat16)

    eye = np.eye(128, dtype=np.float64)
    idnf = np.zeros((13, 128, 128), dtype=np.float64)
    for r in range(6):
        idnf[r] = 2.0 * FE[r] * eye
        idnf[6 + r] = 2.0 * FO[r] * eye
    idnf[12] = eye
    idnb = np.zeros((25, 128, 128), dtype=np.float64)
    idnb[24] = eye
    for r in range(6):
        idnb[r] = 2.0 * FE[r] * eye      # up even
        idnb[6 + r] = 2.0 * FO[r] * eye  # up odd
        idnb[12 + r] = FO[r] * eye       # down, se taps
        idnb[18 + r] = FE[r] * eye       # down, so taps
    idnf = idnf.astype(np.float32)
    idnb = idnb.astype(ml_dtypes.bfloat16)

    al = np.asarray(alphas, dtype=np.float64)      # (6, C) log alpha
    a = np.exp(al)
    c1 = 0.5 / (a + 1e-9)
    snk = np.zeros((NCHUNK, 128, 20), dtype=np.float32)
    snk[:, :, 18] = np.float32(0.25 + 512.0)
    snk[:, :, 19] = np.float32(-np.pi)
    for c in range(NCHUNK):
        for act in range(6):
            snk[c, :, act * 3 + 0] = a[act, c * 128:(c + 1) * 128] / np.pi
            snk[c, :, act * 3 + 1] = -c1[act, c * 128:(c + 1) * 128]
            snk[c, :, act * 3 + 2] = c1[act, c * 128:(c + 1) * 128]

    bia = np.zeros((NCHUNK, 128, 6), dtype=np.float32)
    b1a, b2a = np.asarray(b1), np.asarray(b2)
    for c in range(NCHUNK):
        for it in range(3):
            bia[c, :, it * 2 + 0] = b1a[it, c * 128:(c + 1) * 128]
            bia[c, :, it * 2 + 1] = b2a[it, c * 128:(c + 1) * 128]

    xa = np.asarray(x)
    in_maps = []
    for b in range(B):
        for h in range(2):
            sl = xa[b, 0:TP, :] if h == 0 else xa[b, T - TP:T, :]
            xTc = np.ascontiguousarray(sl.T.astype(np.float32))  # (C, TP)
            in_maps.append({
                "xT": xTc, "wk": wk, "idnf": idnf, "idnb": idnb,
                "snk": snk, "bia": bia,
            })
    return in_maps


def _assemble(results):
    out = np.empty((B, T, C), dtype=np.float32)
    i = 0
    for b in range(B):
        for h in range(2):
            oT = results[i]["out"]          # (C, TP)
            o = np.ascontiguousarray(oT.T)  # (TP, C)
            if h == 0:
                out[b, 0:4096] = o[0:4096]
            else:
                out[b, 4096:T] = o[2 * H:TP]
            i += 1
    return out


def kernel(x, v1, g1, b1, v2, g2, b2, alphas):
    from concourse.bass_utils import run_bass_kernel_spmd

    key = "prog"
    if key not in _PROG_CACHE:
        _PROG_CACHE[key] = _build_program()
    nc = _PROG_CACHE[key]
    in_maps = _host_prep(x, v1, g1, b1, v2, g2, b2, alphas)
    res = run_bass_kernel_spmd(nc, in_maps, core_ids=list(range(8)))
    return _assemble(res.results)
